# revision 33
# baseline (speedup 1.0000x reference)
"""Trainium2 Bass kernel for nn_AGATCellWithMLP (B=8,N=2048,D=64,Q=16,H=2,K=8192).

Sharding: nodes_flat == arange(8192) selects flattened rows 0..8191 == batches
0..3 only; attention for batches 4..7 never reaches the output.  8 cores =
4 batches x 2 n-halves (1024 output rows each), zero collectives.

Per-core pipeline (all matmuls bf16, accumulate f32 in PSUM):
  scores computed transposed  sT[m,n] = (k@qT)  so the softmax numerator p_T
  is directly the lhsT of attn@v; rowsum via a ones-column folded into v;
  leaky-relu+exp on ScalarE batched by activation-table set; adj mask on
  VectorE; hypernetwork gates via G[(q,c),k] = qv_T[q,k]*sel_T[c,k]
  outer-product matmuls, with the qv@b bias folded in as an extra
  contraction tile.  Every DRAM parameter is laid out host-side exactly as
  its SBUF destination so each input is one contiguous DMA.
"""

import numpy as np

B, N, D, Q, H = 8, 2048, 64, 16, 2
C = 2 * D + 1            # 129
C8 = 16
K = 8192
NLOC = 1024              # output rows per core
NCORES = 8
NEG = -9.0e15

_GRAPH_CACHE = {}


# ----------------------------------------------------------------------------
# numpy fallback (correct for arbitrary nodes_flat; slow)
# ----------------------------------------------------------------------------
def _numpy_reference(x, h, query_vectors, adj, nodes_flat,
                     Wq, bq, Wk, bk, Wv, bv,
                     W_r, b_r, W_u, b_u, W_c, b_c):
    x = x.astype(np.float32); h = h.astype(np.float32)
    combined = np.concatenate([x, h], axis=-1)
    q = np.einsum('bnc,hcd->hbnd', combined, Wq) + bq[:, None, None, :]
    k = np.einsum('bnc,hcd->hbnd', combined, Wk) + bk[:, None, None, :]
    v = np.einsum('bnc,hcd->hbnd', combined, Wv) + bv[:, None, None, :]
    comb_attn = np.zeros((B, N, C), np.float32)
    for b in range(B):
        acc = np.zeros((N, C), np.float32)
        for hh in range(H):
            s = (q[hh, b] @ k[hh, b].T) / np.sqrt(np.float32(C8))
            s = np.where(s >= 0, s, 0.2 * s)
            s = np.where(adj[b] == 0, NEG, s)
            s = s - s.max(axis=-1, keepdims=True)
            e = np.exp(s)
            a = e / e.sum(axis=-1, keepdims=True)
            acc += a @ v[hh, b]
        comb_attn[b] = acc / H
    def gate(sel, qv, W, bb):
        t = np.einsum('kc,qco->kqo', sel, W)
        return np.einsum('kq,kqo->ko', qv, t) + qv @ bb
    sel = comb_attn.reshape(-1, C)[nodes_flat]
    r = 1.0 / (1.0 + np.exp(-gate(sel, query_vectors, W_r, b_r)))
    u = 1.0 / (1.0 + np.exp(-gate(sel, query_vectors, W_u, b_u)))
    h_flat = h.reshape(-1, D).copy()
    h_sel = r * h_flat[nodes_flat]
    h_flat[nodes_flat] = h_sel
    comb_new = np.concatenate([x, h_flat.reshape(h.shape)], axis=-1)
    sel2 = comb_new.reshape(-1, C)[nodes_flat]
    cand = np.tanh(gate(sel2, query_vectors, W_c, b_c))
    return ((1.0 - u) * h_sel + u * cand).astype(np.float32)


# ----------------------------------------------------------------------------
# Bass graph builder (one SPMD graph, shapes per core)
# ----------------------------------------------------------------------------
def _build_graph():
    import concourse.bass as bass
    import concourse.bacc as bacc
    import concourse.mybir as mybir
    import concourse.tile as tile
    from concourse import masks
    from contextlib import ExitStack

    bf = mybir.dt.bfloat16
    f32 = mybir.dt.float32
    AF = mybir.ActivationFunctionType
    MUL = mybir.AluOpType.mult
    ADD = mybir.AluOpType.add
    SUB = mybir.AluOpType.subtract

    nc = bacc.Bacc(None, target_bir_lowering=False)

    # ---- DRAM parameters (exact SBUF layouts; one DMA each) -----------------
    wqkva_d  = nc.declare_dram_parameter("wqkva", [128, H, 2 * C8 + 130], bf, isOutput=False)
    wqkvb_d  = nc.declare_dram_parameter("wqkvb", [2, H, 2 * C8 + 130], bf, isOutput=False)
    combT_a  = nc.declare_dram_parameter("combT_a",  [128, N], bf, isOutput=False)
    combT_b  = nc.declare_dram_parameter("combT_b",  [2, N],   bf, isOutput=False)
    combTl_a = nc.declare_dram_parameter("combTl_a", [128, NLOC], bf, isOutput=False)
    combTl_b = nc.declare_dram_parameter("combTl_b", [2, NLOC],   bf, isOutput=False)
    qvT_d    = nc.declare_dram_parameter("qvT",      [Q, NLOC], bf, isOutput=False)
    hloc_d   = nc.declare_dram_parameter("h_loc",    [128, 8, D], f32, isOutput=False)
    hlocT_d  = nc.declare_dram_parameter("h_locT",   [D, NLOC], bf, isOutput=False)
    wru_d    = nc.declare_dram_parameter("wru_flat", [128, 16, 128], bf, isOutput=False)
    wc_d     = nc.declare_dram_parameter("wc_flat",  [128, 16, D], bf, isOutput=False)
    m2bru_d  = nc.declare_dram_parameter("m2b_ru",   [Q, 256], bf, isOutput=False)
    m2bc_d   = nc.declare_dram_parameter("m2b_c",    [Q, 128], bf, isOutput=False)
    adjT_d   = nc.declare_dram_parameter("adjT",     [128, 16, NLOC], bf, isOutput=False)
    qvrep_d  = nc.declare_dram_parameter("qv_rep",   [128, Q, NLOC], bf, isOutput=False)
    out_d    = nc.declare_dram_parameter("out",      [8, 128, D], f32, isOutput=True)

    with ExitStack() as ctx:
        tc = ctx.enter_context(tile.TileContext(nc))
        const = ctx.enter_context(tc.tile_pool(name="const", bufs=1))

        # ---- persistent SBUF tensors + input DMAs (critical first) ----------
        wqkva = const.tile([128, H, 2 * C8 + 130], bf)   # [Wq|Wk|Wv] aug rows 0..127
        wqkvb = const.tile([2, H, 2 * C8 + 130], bf)     # aug rows 128..129
        nc.sync.dma_start(wqkva[:], wqkva_d[:])
        nc.sync.dma_start(wqkvb[:], wqkvb_d[:])
        combTa = const.tile([128, N], bf)
        combTb = const.tile([2, N], bf)
        combTla = const.tile([128, NLOC], bf)
        combTlb = const.tile([2, NLOC], bf)
        nc.sync.dma_start(combTa[:], combT_a[:])
        nc.sync.dma_start(combTb[:], combT_b[:])
        nc.sync.dma_start(combTla[:], combTl_a[:])
        nc.sync.dma_start(combTlb[:], combTl_b[:])

        qvT = const.tile([Q, NLOC], bf)
        nc.sync.dma_start(qvT[:], qvT_d[:])
        hloc = const.tile([128, 8, D], f32)
        nc.sync.dma_start(hloc[:], hloc_d[:])
        hlocT = const.tile([D, NLOC], bf)
        nc.sync.dma_start(hlocT[:], hlocT_d[:])
        wru = const.tile([128, 16, 128], bf)
        nc.sync.dma_start(wru[:], wru_d[:])
        wc = const.tile([128, 16, D], bf)
        nc.sync.dma_start(wc[:], wc_d[:])
        m2bru = const.tile([Q, 256], bf)
        nc.sync.dma_start(m2bru[:], m2bru_d[:])
        m2bc = const.tile([Q, 128], bf)
        nc.sync.dma_start(m2bc[:], m2bc_d[:])

        # adjT and qv_rep share two 16KB/partition slots (disjoint lifetimes);
        # split in halves so the qv_rep DMA can start as soon as the first
        # half of the head-1 masks has consumed adjT.
        big_pool = ctx.enter_context(tc.tile_pool(name="big", bufs=1))
        adjT_h = []
        for bi in range(2):
            at = big_pool.tile([128, 8, NLOC], bf, tag=f"adjqv{bi}")
            nc.sync.dma_start(at[:], adjT_d[:, bi * 8:(bi + 1) * 8, :])
            adjT_h.append(at)

        def adjT(mt):
            return adjT_h[mt // 8][:, mt % 8, :]

        ident = const.tile([128, 128], f32)
        masks.make_identity(nc, ident[:])

        def wslice(hh, lo, hi):
            return wqkva[:, hh, lo:hi], wqkvb[:, hh, lo:hi]

        # ---- projections: qT, kT, v per head --------------------------------
        qTs = const.tile([C8, H, NLOC], bf)
        kTs = const.tile([C8, H, N], bf)
        vs  = const.tile([128, H, 16, 130], bf)

        with tc.tile_pool(name="proj_ps", bufs=2,
                          space=bass.MemorySpace.PSUM) as proj_ps:
            # PE warm-up burst: ~5us of back-to-back dummy matmuls while the
            # input DMAs land, so HAM unthrottles the PE clock (1.2->2.4GHz)
            # before the real matmuls start.
            wz = const.tile([128, 512], bf)
            nc.gpsimd.memset(wz[:], 0.0)
            for _ in range(12):
                wp = proj_ps.tile([128, 512], f32, tag="warm")
                nc.tensor.matmul(wp[:], wz[:, 0:128], wz[:],
                                 start=True, stop=True)
            # q/k projections first so the scores matmuls can start early
            for hh in range(H):
                wka_, wkb_ = wslice(hh, C8, 2 * C8)
                for kchk in range(2):
                    kcs = slice(kchk * NLOC, (kchk + 1) * NLOC)
                    kp = proj_ps.tile([C8, NLOC], f32, tag="kq")
                    for chk in range(2):
                        sl = slice(kchk * NLOC + chk * 512,
                                   kchk * NLOC + (chk + 1) * 512)
                        dl = slice(chk * 512, (chk + 1) * 512)
                        nc.tensor.matmul(kp[:, dl], wka_, combTa[:, sl],
                                         start=True, stop=False)
                        nc.tensor.matmul(kp[:, dl], wkb_, combTb[:, sl],
                                         start=False, stop=True)
                    nc.vector.tensor_copy(kTs[:, hh, kcs], kp[:])

                wqa_, wqb_ = wslice(hh, 0, C8)
                qp = proj_ps.tile([C8, NLOC], f32, tag="kq")
                for chk in range(2):
                    sl = slice(chk * 512, (chk + 1) * 512)
                    nc.tensor.matmul(qp[:, sl], wqa_, combTla[:, sl],
                                     start=True, stop=False)
                    nc.tensor.matmul(qp[:, sl], wqb_, combTlb[:, sl],
                                     start=False, stop=True)
                nc.vector.tensor_copy(qTs[:, hh, :], qp[:])

            pass

        # ---- attention per head --------------------------------------------
        hp0  = const.tile([128, 8, 130], f32)     # head-0: 0.5 * h' / rowsum
        comb = const.tile([128, 8, C], f32)       # mean over heads

        pT_pool = ctx.enter_context(tc.tile_pool(name="pT", bufs=1))
        lr_pool = ctx.enter_context(tc.tile_pool(name="lr", bufs=16))
        ex_pool = ctx.enter_context(tc.tile_pool(name="ex", bufs=3))
        sm_pool = ctx.enter_context(tc.tile_pool(name="small", bufs=2))
        selT = const.tile([128, 8, 128], bf)

        attn_ctx = ExitStack()
        sc_ps = attn_ctx.enter_context(
            tc.tile_pool(name="sc_ps", bufs=2, space=bass.MemorySpace.PSUM))
        hp_ps = attn_ctx.enter_context(
            tc.tile_pool(name="hp_ps", bufs=4, space=bass.MemorySpace.PSUM))

        # Per head: scores + lrelu (batched for one table load), v
        # projections interleaved for PE density, then the exp stream with
        # attn@v chains running mt-outer on 4 PSUM accumulators (j 0..3) so
        # the PE gets ~1us of real work per exp tile; j 4..7 run as a burst
        # right after, overlapping the next head's scores.
        for hh in range(H):
            pT = pT_pool.tile([128, 16, NLOC], bf, tag="pT")
            lrs = []
            wva_, wvb_ = wslice(hh, 2 * C8, 2 * C8 + 130)
            for mt in range(16):
                msl = slice(mt * 128, (mt + 1) * 128)
                sp = sc_ps.tile([128, NLOC], f32, tag="s")
                for chk in range(2):
                    sl = slice(chk * 512, (chk + 1) * 512)
                    nc.tensor.matmul(sp[:, sl], kTs[:, hh, msl],
                                     qTs[:, hh, sl], start=True, stop=True)
                lr = lr_pool.tile([128, NLOC], bf, tag="lr")
                nc.scalar.activation(lr[:], sp[:], AF.Lrelu, alpha=0.2)
                lrs.append(lr)
                # v projection for the same m-tile keeps PE dense
                vp = hp_ps.tile([128, 130], f32, tag="hp")
                nc.tensor.matmul(vp[:], combTa[:, msl], wva_,
                                 start=True, stop=False)
                nc.tensor.matmul(vp[:], combTb[:, msl], wvb_,
                                 start=False, stop=True)
                nc.vector.tensor_copy(vs[:, hh, mt, :], vp[:])

            accums = []
            for _j in range(4):
                acc_t = hp_ps.tile([128, 130], f32, tag="hp")
                accums.append(acc_t)
            for mt in range(16):
                ex = ex_pool.tile([128, NLOC], bf, tag="ex")
                nc.scalar.activation(ex[:], lrs[mt][:], AF.Exp)
                nc.vector.tensor_tensor(pT[:, mt, :], ex[:], adjT(mt),
                                        op=MUL)
                for j in range(4):
                    jsl = slice(j * 128, (j + 1) * 128)
                    nc.tensor.matmul(accums[j][:], pT[:, mt, jsl],
                                     vs[:, hh, mt, :],
                                     start=(mt == 0), stop=(mt == 15))

            def _combine(hp_ap, j):
                rs = sm_pool.tile([128, 1], f32, tag="rs")
                nc.vector.reciprocal(rs[:], hp_ap[:, 129:130])
                if hh == 0:
                    nc.vector.tensor_scalar(hp0[:, j, 0:C], hp_ap[:, 0:C],
                                            rs[:], 0.5, op0=MUL, op1=MUL)
                else:
                    t1 = sm_pool.tile([128, C], f32, tag="t1")
                    nc.vector.tensor_scalar(t1[:], hp_ap[:, 0:C], rs[:], 0.5,
                                            op0=MUL, op1=MUL)
                    nc.vector.tensor_tensor(comb[:, j, :], hp0[:, j, 0:C],
                                            t1[:], op=ADD)
                    tpj = hp_ps.tile([128, 130], f32, tag="hp")
                    nc.tensor.transpose(tpj[:, 0:128], comb[:, j, 0:128],
                                        ident[:])
                    nc.vector.tensor_copy(selT[:, j, :], tpj[:, 0:128])

            for j in range(4):
                _combine(accums[j][:], j)
            for j in range(4, 8):
                jsl = slice(j * 128, (j + 1) * 128)
                hp = hp_ps.tile([128, 130], f32, tag="hp")
                for mt in range(16):
                    nc.tensor.matmul(hp[:], pT[:, mt, jsl], vs[:, hh, mt, :],
                                     start=(mt == 0), stop=(mt == 15))
                _combine(hp[:], j)

        attn_ctx.close()

        tp_ps = ctx.enter_context(
            tc.tile_pool(name="tp_ps", bufs=2, space=bass.MemorySpace.PSUM))

        def keep_warm(n=2):
            # HAM re-throttles the PE after ~3.4us of low activity; trickle
            # dummy matmuls into ACT/DVE-paced stretches to hold 2.4GHz.
            for _ in range(n):
                wp = tp_ps.tile([128, 128], f32, tag="tp")
                nc.tensor.matmul(wp[:], wz[:, 0:128], wz[:, 0:128],
                                 start=True, stop=True)

        # ---- gates r,u ------------------------------------------------------
        # qv_rep reuses adjT's half-slots (masks consumed them above)
        qvrep_h = []
        for bi in range(2):
            qr = big_pool.tile([128, 8, NLOC], bf, tag=f"adjqv{bi}")
            nc.sync.dma_start(qr[:], qvrep_d[:, bi * 8:(bi + 1) * 8, :])
            qvrep_h.append(qr)

        def qvrep(qt):
            return qvrep_h[qt // 8][:, qt % 8, :]

        G_pool = ctx.enter_context(tc.tile_pool(name="G", bufs=1))
        g_ps = ctx.enter_context(
            tc.tile_pool(name="g_ps", bufs=2, space=bass.MemorySpace.PSUM))
        m2_ps = ctx.enter_context(
            tc.tile_pool(name="m2_ps", bufs=2, space=bass.MemorySpace.PSUM))

        ru   = const.tile([128, 8, 128], f32)   # sigmoid outputs: r | u
        hsel = const.tile([128, 8, D], f32)
        rTb  = const.tile([D, 8, 128], bf)

        selT_flat = selT[:, :, :].rearrange("p a b -> p (a b)")
        for gc in range(2):
            gsl = slice(gc * 512, (gc + 1) * 512)
            G = G_pool.tile([128, Q, 512], bf, tag="G")
            for qt in range(Q):
                eng = nc.gpsimd if qt % 4 == 3 else nc.vector
                eng.tensor_tensor(G[:, qt, :], selT_flat[:, gsl],
                                  qvrep(qt)[:, gsl], op=MUL)
            for kt in range(4 * gc, 4 * gc + 4):
                ksl = slice(kt * 128, (kt + 1) * 128)
                lsl = slice((kt - 4 * gc) * 128, (kt - 4 * gc + 1) * 128)
                gp = g_ps.tile([128, 128], f32, tag="g")
                for qt in range(Q):
                    nc.tensor.matmul(gp[:], G[:, qt, lsl], wru[:, qt, :],
                                     start=(qt == 0), stop=False)
                # bias qv@[b_r|b_u] as an extra contraction tile
                nc.tensor.matmul(gp[:], qvT[:, ksl], m2bru[:, 128:256],
                                 start=False, stop=True)
                m2 = m2_ps.tile([128, 128], f32, tag="m2")
                nc.tensor.matmul(m2[:], qvT[:, ksl], m2bru[:, 0:128],
                                 start=True, stop=True)
                a = sm_pool.tile([128, 128], f32, tag="ga")
                nc.vector.tensor_scalar(a[:], m2[:], comb[:, kt, 128:129],
                                        None, op0=MUL)
                pre = sm_pool.tile([128, 128], f32, tag="gp")
                nc.vector.tensor_tensor(pre[:], a[:], gp[:], op=ADD)
                nc.scalar.activation(ru[:, kt, :], pre[:], AF.Sigmoid)
                nc.vector.tensor_tensor(hsel[:, kt, :], ru[:, kt, 0:D],
                                        hloc[:, kt, :], op=MUL)
                # r transpose for sel2T, interleaved
                tp = tp_ps.tile([D, 128], f32, tag="tp")
                nc.tensor.transpose(tp[:], ru[:, kt, 0:D], ident[:])
                nc.vector.tensor_copy(rTb[:, kt, :], tp[:])

        # ---- gate c, pipelined per 512-chunk after ru ----------------------
        rTb_flat = rTb[:, :, :].rearrange("p a b -> p (a b)")
        rh = const.tile([D, NLOC], bf)
        sel2T = const.tile([128, NLOC], bf)
        cand = const.tile([128, 8, D], f32)
        out_sb = const.tile([128, 8, D], f32)
        for gc in range(2):
            gsl = slice(gc * 512, (gc + 1) * 512)
            # sel2T chunk: [x_T(65) | (r*h)T(63)]
            nc.vector.tensor_tensor(rh[:, gsl], rTb_flat[:, gsl],
                                    hlocT[:, gsl], op=MUL)
            nc.vector.tensor_copy(sel2T[0:65, gsl], combTla[0:65, gsl])
            nc.sync.dma_start(sel2T[65:128, gsl], rh[0:63, gsl])
            G2 = G_pool.tile([128, Q, 512], bf, tag="G")
            for qt in range(Q):
                eng = nc.gpsimd if qt % 4 == 3 else nc.vector
                eng.tensor_tensor(G2[:, qt, :], sel2T[:, gsl],
                                  qvrep(qt)[:, gsl], op=MUL)
            for kt in range(4 * gc, 4 * gc + 4):
                ksl = slice(kt * 128, (kt + 1) * 128)
                lsl = slice((kt - 4 * gc) * 128, (kt - 4 * gc + 1) * 128)
                gp = g_ps.tile([128, D], f32, tag="g")
                for qt in range(Q):
                    nc.tensor.matmul(gp[:], G2[:, qt, lsl], wc[:, qt, :],
                                     start=(qt == 0), stop=False)
                nc.tensor.matmul(gp[:], qvT[:, ksl], m2bc[:, D:128],
                                 start=False, stop=True)
                m2 = m2_ps.tile([128, D], f32, tag="m2")
                nc.tensor.matmul(m2[:], qvT[:, ksl], m2bc[:, 0:D],
                                 start=True, stop=True)
                a = sm_pool.tile([128, D], f32, tag="ca")
                # sel2 col 128 is h_sel[:, 63]
                nc.vector.tensor_scalar(a[:], m2[:], hsel[:, kt, 63:64], None,
                                        op0=MUL)
                pre = sm_pool.tile([128, D], f32, tag="cp")
                nc.vector.tensor_tensor(pre[:], a[:], gp[:], op=ADD)
                nc.scalar.activation(cand[:, kt, :], pre[:], AF.Tanh)
                # output: (1-u)*h_sel + u*cand
                t = sm_pool.tile([128, D], f32, tag="o1")
                nc.vector.tensor_tensor(t[:], cand[:, kt, :], hsel[:, kt, :],
                                        op=SUB)
                t2 = sm_pool.tile([128, D], f32, tag="o2")
                nc.vector.tensor_tensor(t2[:], t[:], ru[:, kt, 64:128], op=MUL)
                nc.vector.tensor_tensor(out_sb[:, kt, :], t2[:],
                                        hsel[:, kt, :], op=ADD)
                nc.sync.dma_start(out_d[kt], out_sb[:, kt, :])

    if not nc.is_finalized():
        nc.finalize()
    return nc


def _get_graph():
    if "nc" not in _GRAPH_CACHE:
        _GRAPH_CACHE["nc"] = _build_graph()
    return _GRAPH_CACHE["nc"]


# ----------------------------------------------------------------------------
# host-side input prep
# ----------------------------------------------------------------------------
def _prep_in_maps(x, h, query_vectors, adj,
                  Wq, bq, Wk, bk, Wv, bv,
                  W_r, b_r, W_u, b_u, W_c, b_c):
    import ml_dtypes
    bf = ml_dtypes.bfloat16

    scale = 1.0 / np.sqrt(np.float32(C8))

    # packed per-head augmented projection weights: [130, Wq(16)|Wk(16)|Wv2(130)]
    wqkv = np.zeros((H, 130, 2 * C8 + 130), np.float32)
    for hh in range(H):
        wqkv[hh, 0:C, 0:C8] = Wq[hh] * scale
        wqkv[hh, C, 0:C8] = bq[hh] * scale
        wqkv[hh, 0:C, C8:2 * C8] = Wk[hh]
        wqkv[hh, C, C8:2 * C8] = bk[hh]
        wqkv[hh, 0:C, 2 * C8:2 * C8 + C] = Wv[hh]
        wqkv[hh, C, 2 * C8:2 * C8 + C] = bv[hh]
        wqkv[hh, C, 2 * C8 + C] = 1.0          # ones-column -> rowsum
    wqkv = np.ascontiguousarray(wqkv.transpose(1, 0, 2))      # [130, H, 162]

    # gate weights, flattened (q-major over (q, c)) for c = 0..127,
    # reshaped to the SBUF tile layout [128(c), 16(q), outdim]
    wru_flat = np.concatenate([W_r[:, 0:128, :], W_u[:, 0:128, :]], axis=2)
    wru_flat = np.ascontiguousarray(wru_flat.transpose(1, 0, 2))  # [128, 16, 128]
    wc_flat = np.ascontiguousarray(W_c[:, 0:128, :].transpose(1, 0, 2))
    m2b_ru = np.concatenate(
        [W_r[:, 128, :], W_u[:, 128, :], b_r, b_u], axis=1)       # [16, 256]
    m2b_c = np.concatenate([W_c[:, 128, :], b_c], axis=1)         # [16, 128]

    shared = {
        "wqkva": wqkv[0:128].astype(bf), "wqkvb": wqkv[128:130].astype(bf),
        "wru_flat": wru_flat.astype(bf), "wc_flat": wc_flat.astype(bf),
        "m2b_ru": m2b_ru.astype(bf), "m2b_c": m2b_c.astype(bf),
    }

    in_maps = []
    for core in range(NCORES):
        b, half = core // 2, core % 2
        n0 = half * NLOC
        g0 = b * N + n0

        combined = np.concatenate(
            [x[b], h[b], np.ones((N, 1), np.float32)], axis=1)    # [N, 130]
        combT = np.ascontiguousarray(combined.T)                  # [130, N]
        qvT = np.ascontiguousarray(query_vectors[g0:g0 + NLOC].T) # [16, 1024]
        # adjT[p, mt, k] = adj[b][n0+k, mt*128+p]
        adjT = np.ascontiguousarray(
            adj[b].T[:, n0:n0 + NLOC].reshape(16, 128, NLOC)
            .transpose(1, 0, 2)).astype(np.float32)               # [128,16,1024]
        qvrep = np.ascontiguousarray(
            np.broadcast_to(qvT[None, :, :], (128, Q, NLOC)))     # [128,16,1024]

        m = {
            "combT_a": combT[0:128].astype(bf),
            "combT_b": combT[128:130].astype(bf),
            "combTl_a": np.ascontiguousarray(combT[0:128, n0:n0 + NLOC]).astype(bf),
            "combTl_b": np.ascontiguousarray(combT[128:130, n0:n0 + NLOC]).astype(bf),
            "adjT": adjT.astype(bf),
            "qv_rep": qvrep.astype(bf),
            "qvT": qvT.astype(bf),
            "h_loc": np.ascontiguousarray(
                h[b, n0:n0 + NLOC].reshape(8, 128, D).transpose(1, 0, 2)),
            "h_locT": np.ascontiguousarray(h[b, n0:n0 + NLOC].T).astype(bf),
        }
        m.update(shared)
        in_maps.append(m)
    return in_maps


# ----------------------------------------------------------------------------
# entry point
# ----------------------------------------------------------------------------
def kernel(x, h, query_vectors, adj, nodes_flat,
           Wq, bq, Wk, bk, Wv, bv,
           W_r, b_r, W_u, b_u, W_c, b_c, _trace=False):
    args = dict(x=np.asarray(x, np.float32), h=np.asarray(h, np.float32),
                query_vectors=np.asarray(query_vectors, np.float32),
                adj=np.asarray(adj), nodes_flat=np.asarray(nodes_flat),
                Wq=np.asarray(Wq, np.float32), bq=np.asarray(bq, np.float32),
                Wk=np.asarray(Wk, np.float32), bk=np.asarray(bk, np.float32),
                Wv=np.asarray(Wv, np.float32), bv=np.asarray(bv, np.float32),
                W_r=np.asarray(W_r, np.float32), b_r=np.asarray(b_r, np.float32),
                W_u=np.asarray(W_u, np.float32), b_u=np.asarray(b_u, np.float32),
                W_c=np.asarray(W_c, np.float32), b_c=np.asarray(b_c, np.float32))

    if not np.array_equal(args["nodes_flat"].ravel(),
                          np.arange(K, dtype=np.int64)):
        return _numpy_reference(**args)

    from concourse.bass_utils import run_bass_kernel_spmd

    nc = _get_graph()
    in_maps = _prep_in_maps(
        args["x"], args["h"], args["query_vectors"], args["adj"],
        args["Wq"], args["bq"], args["Wk"], args["bk"], args["Wv"], args["bv"],
        args["W_r"], args["b_r"], args["W_u"], args["b_u"],
        args["W_c"], args["b_c"])

    res = run_bass_kernel_spmd(nc, in_maps, core_ids=list(range(NCORES)),
                               trace=_trace)
    out = np.concatenate(
        [np.asarray(res.results[i]["out"], np.float32).reshape(NLOC, D)
         for i in range(NCORES)], axis=0)
    if _trace:
        kernel.last_exec_time_ns = res.exec_time_ns
    return out


# revision 34
# speedup vs baseline: 1.2759x; 1.2759x over previous
"""Trainium2 Bass kernel for nn_AGATCellWithMLP (B=8,N=2048,D=64,Q=16,H=2,K=8192).

Sharding: nodes_flat == arange(8192) selects flattened rows 0..8191 == batches
0..3 only; attention for batches 4..7 never reaches the output.  8 cores =
4 batches x 2 n-halves (1024 output rows each), zero collectives.

Per-core pipeline (all matmuls bf16, accumulate f32 in PSUM):
  scores computed transposed  sT[m,n] = (k@qT)  so the softmax numerator p_T
  is directly the lhsT of attn@v; rowsum via a ones-column folded into v;
  leaky-relu+exp on ScalarE batched by activation-table set; adj mask on
  VectorE; hypernetwork gates via G[(q,c),k] = qv_T[q,k]*sel_T[c,k]
  outer-product matmuls, with the qv@b bias folded in as an extra
  contraction tile.  Every DRAM parameter is laid out host-side exactly as
  its SBUF destination so each input is one contiguous DMA.
"""

import numpy as np

B, N, D, Q, H = 8, 2048, 64, 16, 2
C = 2 * D + 1            # 129
C8 = 16
K = 8192
NLOC = 1024              # output rows per core
NCORES = 8
NEG = -9.0e15

_GRAPH_CACHE = {}


# ----------------------------------------------------------------------------
# numpy fallback (correct for arbitrary nodes_flat; slow)
# ----------------------------------------------------------------------------
def _numpy_reference(x, h, query_vectors, adj, nodes_flat,
                     Wq, bq, Wk, bk, Wv, bv,
                     W_r, b_r, W_u, b_u, W_c, b_c):
    x = x.astype(np.float32); h = h.astype(np.float32)
    combined = np.concatenate([x, h], axis=-1)
    q = np.einsum('bnc,hcd->hbnd', combined, Wq) + bq[:, None, None, :]
    k = np.einsum('bnc,hcd->hbnd', combined, Wk) + bk[:, None, None, :]
    v = np.einsum('bnc,hcd->hbnd', combined, Wv) + bv[:, None, None, :]
    comb_attn = np.zeros((B, N, C), np.float32)
    for b in range(B):
        acc = np.zeros((N, C), np.float32)
        for hh in range(H):
            s = (q[hh, b] @ k[hh, b].T) / np.sqrt(np.float32(C8))
            s = np.where(s >= 0, s, 0.2 * s)
            s = np.where(adj[b] == 0, NEG, s)
            s = s - s.max(axis=-1, keepdims=True)
            e = np.exp(s)
            a = e / e.sum(axis=-1, keepdims=True)
            acc += a @ v[hh, b]
        comb_attn[b] = acc / H
    def gate(sel, qv, W, bb):
        t = np.einsum('kc,qco->kqo', sel, W)
        return np.einsum('kq,kqo->ko', qv, t) + qv @ bb
    sel = comb_attn.reshape(-1, C)[nodes_flat]
    r = 1.0 / (1.0 + np.exp(-gate(sel, query_vectors, W_r, b_r)))
    u = 1.0 / (1.0 + np.exp(-gate(sel, query_vectors, W_u, b_u)))
    h_flat = h.reshape(-1, D).copy()
    h_sel = r * h_flat[nodes_flat]
    h_flat[nodes_flat] = h_sel
    comb_new = np.concatenate([x, h_flat.reshape(h.shape)], axis=-1)
    sel2 = comb_new.reshape(-1, C)[nodes_flat]
    cand = np.tanh(gate(sel2, query_vectors, W_c, b_c))
    return ((1.0 - u) * h_sel + u * cand).astype(np.float32)


# ----------------------------------------------------------------------------
# Bass graph builder (one SPMD graph, shapes per core)
# ----------------------------------------------------------------------------
def _build_graph():
    import concourse.bass as bass
    import concourse.bacc as bacc
    import concourse.mybir as mybir
    import concourse.tile as tile
    from concourse import masks
    from contextlib import ExitStack

    bf = mybir.dt.bfloat16
    f32 = mybir.dt.float32
    AF = mybir.ActivationFunctionType
    MUL = mybir.AluOpType.mult
    ADD = mybir.AluOpType.add
    SUB = mybir.AluOpType.subtract

    nc = bacc.Bacc(None, target_bir_lowering=False)

    # ---- DRAM parameters (exact SBUF layouts; one DMA each) -----------------
    wqkva_d  = nc.declare_dram_parameter("wqkva", [128, H, 2 * C8 + 130], bf, isOutput=False)
    wqkvb_d  = nc.declare_dram_parameter("wqkvb", [2, H, 2 * C8 + 130], bf, isOutput=False)
    combT_a  = nc.declare_dram_parameter("combT_a",  [128, N], bf, isOutput=False)
    combT_b  = nc.declare_dram_parameter("combT_b",  [2, N],   bf, isOutput=False)
    combTl_a = nc.declare_dram_parameter("combTl_a", [128, NLOC], bf, isOutput=False)
    combTl_b = nc.declare_dram_parameter("combTl_b", [2, NLOC],   bf, isOutput=False)
    qvT_d    = nc.declare_dram_parameter("qvT",      [Q, NLOC], bf, isOutput=False)
    hloc_d   = nc.declare_dram_parameter("h_loc",    [128, 8, D], f32, isOutput=False)
    hlocT_d  = nc.declare_dram_parameter("h_locT",   [D, NLOC], bf, isOutput=False)
    wru_d    = nc.declare_dram_parameter("wru_flat", [128, 16, 128], bf, isOutput=False)
    wc_d     = nc.declare_dram_parameter("wc_flat",  [128, 16, D], bf, isOutput=False)
    m2bru_d  = nc.declare_dram_parameter("m2b_ru",   [Q, 256], bf, isOutput=False)
    m2bc_d   = nc.declare_dram_parameter("m2b_c",    [Q, 128], bf, isOutput=False)
    adjT_d   = nc.declare_dram_parameter("adjT",     [128, 16, NLOC], bf, isOutput=False)
    qvrep_d  = nc.declare_dram_parameter("qv_rep",   [128, Q, NLOC], bf, isOutput=False)
    out_d    = nc.declare_dram_parameter("out",      [8, 128, D], f32, isOutput=True)

    with ExitStack() as ctx:
        tc = ctx.enter_context(tile.TileContext(nc))
        const = ctx.enter_context(tc.tile_pool(name="const", bufs=1))

        # ---- persistent SBUF tensors + input DMAs (critical first) ----------
        wqkva = const.tile([128, H, 2 * C8 + 130], bf)   # [Wq|Wk|Wv] aug rows 0..127
        wqkvb = const.tile([2, H, 2 * C8 + 130], bf)     # aug rows 128..129
        nc.sync.dma_start(wqkva[:], wqkva_d[:])
        nc.sync.dma_start(wqkvb[:], wqkvb_d[:])
        combTa = const.tile([128, N], bf)
        combTb = const.tile([2, N], bf)
        combTla = const.tile([128, NLOC], bf)
        combTlb = const.tile([2, NLOC], bf)
        nc.sync.dma_start(combTa[:], combT_a[:])
        nc.sync.dma_start(combTb[:], combT_b[:])
        nc.sync.dma_start(combTla[:], combTl_a[:])
        nc.sync.dma_start(combTlb[:], combTl_b[:])

        qvT = const.tile([Q, NLOC], bf)
        nc.sync.dma_start(qvT[:], qvT_d[:])
        hloc = const.tile([128, 8, D], f32)
        nc.sync.dma_start(hloc[:], hloc_d[:])
        hlocT = const.tile([D, NLOC], bf)
        nc.sync.dma_start(hlocT[:], hlocT_d[:])
        wru = const.tile([128, 16, 128], bf)
        nc.sync.dma_start(wru[:], wru_d[:])
        wc = const.tile([128, 16, D], bf)
        nc.sync.dma_start(wc[:], wc_d[:])
        m2bru = const.tile([Q, 256], bf)
        nc.sync.dma_start(m2bru[:], m2bru_d[:])
        m2bc = const.tile([Q, 128], bf)
        nc.sync.dma_start(m2bc[:], m2bc_d[:])

        # adjT and qv_rep share two 16KB/partition slots (disjoint lifetimes);
        # split in halves so the qv_rep DMA can start as soon as the first
        # half of the head-1 masks has consumed adjT.
        big_pool = ctx.enter_context(tc.tile_pool(name="big", bufs=1))
        adjT_h = []
        for bi in range(2):
            at = big_pool.tile([128, 8, NLOC], bf, tag=f"adjqv{bi}")
            nc.sync.dma_start(at[:], adjT_d[:, bi * 8:(bi + 1) * 8, :])
            adjT_h.append(at)

        def adjT(mt):
            return adjT_h[mt // 8][:, mt % 8, :]

        ident = const.tile([128, 128], f32)
        masks.make_identity(nc, ident[:])

        def wslice(hh, lo, hi):
            return wqkva[:, hh, lo:hi], wqkvb[:, hh, lo:hi]

        # ---- projections: qT, kT, v per head --------------------------------
        qTs = const.tile([C8, H, NLOC], bf)
        kTs = const.tile([C8, H, N], bf)
        vs  = const.tile([128, H, 16, 130], bf)

        with tc.tile_pool(name="proj_ps", bufs=2,
                          space=bass.MemorySpace.PSUM) as proj_ps:
            # PE warm-up burst: ~5us of back-to-back dummy matmuls while the
            # input DMAs land, so HAM unthrottles the PE clock (1.2->2.4GHz)
            # before the real matmuls start.
            wz = const.tile([128, 512], bf)
            nc.gpsimd.memset(wz[:], 0.0)
            for _ in range(12):
                wp = proj_ps.tile([128, 512], f32, tag="warm")
                nc.tensor.matmul(wp[:], wz[:, 0:128], wz[:],
                                 start=True, stop=True)
            # q/k projections first so the scores matmuls can start early
            for hh in range(H):
                wka_, wkb_ = wslice(hh, C8, 2 * C8)
                for kchk in range(2):
                    kcs = slice(kchk * NLOC, (kchk + 1) * NLOC)
                    kp = proj_ps.tile([C8, NLOC], f32, tag="kq")
                    for chk in range(2):
                        sl = slice(kchk * NLOC + chk * 512,
                                   kchk * NLOC + (chk + 1) * 512)
                        dl = slice(chk * 512, (chk + 1) * 512)
                        nc.tensor.matmul(kp[:, dl], wka_, combTa[:, sl],
                                         start=True, stop=False)
                        nc.tensor.matmul(kp[:, dl], wkb_, combTb[:, sl],
                                         start=False, stop=True)
                    nc.vector.tensor_copy(kTs[:, hh, kcs], kp[:])

                wqa_, wqb_ = wslice(hh, 0, C8)
                qp = proj_ps.tile([C8, NLOC], f32, tag="kq")
                for chk in range(2):
                    sl = slice(chk * 512, (chk + 1) * 512)
                    nc.tensor.matmul(qp[:, sl], wqa_, combTla[:, sl],
                                     start=True, stop=False)
                    nc.tensor.matmul(qp[:, sl], wqb_, combTlb[:, sl],
                                     start=False, stop=True)
                nc.vector.tensor_copy(qTs[:, hh, :], qp[:])

            pass

        # ---- attention per head --------------------------------------------
        hp0  = const.tile([128, 8, 130], f32)     # head-0: 0.5 * h' / rowsum
        comb = const.tile([128, 8, C], f32)       # mean over heads

        pT_pool = ctx.enter_context(tc.tile_pool(name="pT", bufs=1))
        lr_pool = ctx.enter_context(tc.tile_pool(name="lr", bufs=16))
        ex_pool = ctx.enter_context(tc.tile_pool(name="ex", bufs=3))
        sm_pool = ctx.enter_context(tc.tile_pool(name="small", bufs=2))
        selT = const.tile([128, 8, 128], bf)

        attn_ctx = ExitStack()
        sc_ps = attn_ctx.enter_context(
            tc.tile_pool(name="sc_ps", bufs=2, space=bass.MemorySpace.PSUM))
        hp_ps = attn_ctx.enter_context(
            tc.tile_pool(name="hp_ps", bufs=4, space=bass.MemorySpace.PSUM))

        # Per head: scores + lrelu (batched for one table load), v
        # projections interleaved for PE density, then the exp stream with
        # attn@v chains running mt-outer on 4 PSUM accumulators (j 0..3) so
        # the PE gets ~1us of real work per exp tile; j 4..7 run as a burst
        # right after, overlapping the next head's scores.
        for hh in range(H):
            pT = pT_pool.tile([128, 16, NLOC], bf, tag="pT")
            lrs = []
            wva_, wvb_ = wslice(hh, 2 * C8, 2 * C8 + 130)
            for mt in range(16):
                msl = slice(mt * 128, (mt + 1) * 128)
                sp = sc_ps.tile([128, NLOC], f32, tag="s")
                for chk in range(2):
                    sl = slice(chk * 512, (chk + 1) * 512)
                    nc.tensor.matmul(sp[:, sl], kTs[:, hh, msl],
                                     qTs[:, hh, sl], start=True, stop=True)
                lr = lr_pool.tile([128, NLOC], bf, tag="lr")
                nc.scalar.activation(lr[:], sp[:], AF.Lrelu, alpha=0.2)
                lrs.append(lr)
                # v projection for the same m-tile keeps PE dense
                vp = hp_ps.tile([128, 130], f32, tag="hp")
                nc.tensor.matmul(vp[:], combTa[:, msl], wva_,
                                 start=True, stop=False)
                nc.tensor.matmul(vp[:], combTb[:, msl], wvb_,
                                 start=False, stop=True)
                nc.vector.tensor_copy(vs[:, hh, mt, :], vp[:])

            accums = []
            for _j in range(4):
                acc_t = hp_ps.tile([128, 130], f32, tag="hp")
                accums.append(acc_t)
            for mt in range(16):
                ex = ex_pool.tile([128, NLOC], bf, tag="ex")
                nc.scalar.activation(ex[:], lrs[mt][:], AF.Exp)
                nc.vector.tensor_tensor(pT[:, mt, :], ex[:], adjT(mt),
                                        op=MUL)
                for j in range(4):
                    jsl = slice(j * 128, (j + 1) * 128)
                    nc.tensor.matmul(accums[j][:], pT[:, mt, jsl],
                                     vs[:, hh, mt, :],
                                     start=(mt == 0), stop=(mt == 15))

            def _combine(hp_ap, j):
                rs = sm_pool.tile([128, 1], f32, tag="rs")
                nc.vector.reciprocal(rs[:], hp_ap[:, 129:130])
                if hh == 0:
                    nc.vector.tensor_scalar(hp0[:, j, 0:C], hp_ap[:, 0:C],
                                            rs[:], 0.5, op0=MUL, op1=MUL)
                else:
                    t1 = sm_pool.tile([128, C], f32, tag="t1")
                    nc.vector.tensor_scalar(t1[:], hp_ap[:, 0:C], rs[:], 0.5,
                                            op0=MUL, op1=MUL)
                    nc.vector.tensor_tensor(comb[:, j, :], hp0[:, j, 0:C],
                                            t1[:], op=ADD)
                    tpj = hp_ps.tile([128, 130], f32, tag="hp")
                    nc.tensor.transpose(tpj[:, 0:128], comb[:, j, 0:128],
                                        ident[:])
                    nc.vector.tensor_copy(selT[:, j, :], tpj[:, 0:128])

            for j in range(4):
                _combine(accums[j][:], j)
            for j in range(4, 8):
                jsl = slice(j * 128, (j + 1) * 128)
                hp = hp_ps.tile([128, 130], f32, tag="hp")
                for mt in range(16):
                    nc.tensor.matmul(hp[:], pT[:, mt, jsl], vs[:, hh, mt, :],
                                     start=(mt == 0), stop=(mt == 15))
                _combine(hp[:], j)

        attn_ctx.close()

        tp_ps = ctx.enter_context(
            tc.tile_pool(name="tp_ps", bufs=2, space=bass.MemorySpace.PSUM))

        def keep_warm(n=2):
            # HAM re-throttles the PE after ~3.4us of low activity; trickle
            # dummy matmuls into ACT/DVE-paced stretches to hold 2.4GHz.
            for _ in range(n):
                wp = tp_ps.tile([128, 128], f32, tag="tp")
                nc.tensor.matmul(wp[:], wz[:, 0:128], wz[:, 0:128],
                                 start=True, stop=True)

        # ---- gates r,u ------------------------------------------------------
        # qv_rep reuses adjT's half-slots (masks consumed them above)
        qvrep_h = []
        for bi in range(2):
            qr = big_pool.tile([128, 8, NLOC], bf, tag=f"adjqv{bi}")
            nc.sync.dma_start(qr[:], qvrep_d[:, bi * 8:(bi + 1) * 8, :])
            qvrep_h.append(qr)

        def qvrep(qt):
            return qvrep_h[qt // 8][:, qt % 8, :]

        G_pool = ctx.enter_context(tc.tile_pool(name="G", bufs=1))
        g_ps = ctx.enter_context(
            tc.tile_pool(name="g_ps", bufs=2, space=bass.MemorySpace.PSUM))
        m2_ps = ctx.enter_context(
            tc.tile_pool(name="m2_ps", bufs=2, space=bass.MemorySpace.PSUM))

        ru   = const.tile([128, 8, 128], f32)   # sigmoid outputs: r | u
        hsel = const.tile([128, 8, D], f32)
        rTb  = const.tile([D, 8, 128], bf)

        selT_flat = selT[:, :, :].rearrange("p a b -> p (a b)")
        for gc in range(2):
            gsl = slice(gc * 512, (gc + 1) * 512)
            G = G_pool.tile([128, Q, 512], bf, tag="G")
            for qt in range(Q):
                nc.vector.tensor_tensor(G[:, qt, :], selT_flat[:, gsl],
                                        qvrep(qt)[:, gsl], op=MUL)
            for kt in range(4 * gc, 4 * gc + 4):
                ksl = slice(kt * 128, (kt + 1) * 128)
                lsl = slice((kt - 4 * gc) * 128, (kt - 4 * gc + 1) * 128)
                gp = g_ps.tile([128, 128], f32, tag="g")
                for qt in range(Q):
                    nc.tensor.matmul(gp[:], G[:, qt, lsl], wru[:, qt, :],
                                     start=(qt == 0), stop=False)
                # bias qv@[b_r|b_u] as an extra contraction tile
                nc.tensor.matmul(gp[:], qvT[:, ksl], m2bru[:, 128:256],
                                 start=False, stop=True)
                m2 = m2_ps.tile([128, 128], f32, tag="m2")
                nc.tensor.matmul(m2[:], qvT[:, ksl], m2bru[:, 0:128],
                                 start=True, stop=True)
                a = sm_pool.tile([128, 128], f32, tag="ga")
                nc.vector.tensor_scalar(a[:], m2[:], comb[:, kt, 128:129],
                                        None, op0=MUL)
                pre = sm_pool.tile([128, 128], f32, tag="gp")
                nc.vector.tensor_tensor(pre[:], a[:], gp[:], op=ADD)
                nc.scalar.activation(ru[:, kt, :], pre[:], AF.Sigmoid)
                nc.vector.tensor_tensor(hsel[:, kt, :], ru[:, kt, 0:D],
                                        hloc[:, kt, :], op=MUL)
                # r transpose for sel2T, interleaved
                tp = tp_ps.tile([D, 128], f32, tag="tp")
                nc.tensor.transpose(tp[:], ru[:, kt, 0:D], ident[:])
                nc.vector.tensor_copy(rTb[:, kt, :], tp[:])

        # ---- gate c, pipelined per 512-chunk after ru ----------------------
        rTb_flat = rTb[:, :, :].rearrange("p a b -> p (a b)")
        rh = const.tile([D, NLOC], bf)
        sel2T = const.tile([128, NLOC], bf)
        cand = const.tile([128, 8, D], f32)
        out_sb = const.tile([128, 8, D], f32)
        for gc in range(2):
            gsl = slice(gc * 512, (gc + 1) * 512)
            # sel2T chunk: [x_T(65) | (r*h)T(63)]
            nc.vector.tensor_tensor(rh[:, gsl], rTb_flat[:, gsl],
                                    hlocT[:, gsl], op=MUL)
            nc.vector.tensor_copy(sel2T[0:65, gsl], combTla[0:65, gsl])
            nc.sync.dma_start(sel2T[65:128, gsl], rh[0:63, gsl])
            G2 = G_pool.tile([128, Q, 512], bf, tag="G")
            for qt in range(Q):
                nc.vector.tensor_tensor(G2[:, qt, :], sel2T[:, gsl],
                                        qvrep(qt)[:, gsl], op=MUL)
            for kt in range(4 * gc, 4 * gc + 4):
                ksl = slice(kt * 128, (kt + 1) * 128)
                lsl = slice((kt - 4 * gc) * 128, (kt - 4 * gc + 1) * 128)
                gp = g_ps.tile([128, D], f32, tag="g")
                for qt in range(Q):
                    nc.tensor.matmul(gp[:], G2[:, qt, lsl], wc[:, qt, :],
                                     start=(qt == 0), stop=False)
                nc.tensor.matmul(gp[:], qvT[:, ksl], m2bc[:, D:128],
                                 start=False, stop=True)
                m2 = m2_ps.tile([128, D], f32, tag="m2")
                nc.tensor.matmul(m2[:], qvT[:, ksl], m2bc[:, 0:D],
                                 start=True, stop=True)
                a = sm_pool.tile([128, D], f32, tag="ca")
                # sel2 col 128 is h_sel[:, 63]
                nc.vector.tensor_scalar(a[:], m2[:], hsel[:, kt, 63:64], None,
                                        op0=MUL)
                pre = sm_pool.tile([128, D], f32, tag="cp")
                nc.vector.tensor_tensor(pre[:], a[:], gp[:], op=ADD)
                nc.scalar.activation(cand[:, kt, :], pre[:], AF.Tanh)
                # output: (1-u)*h_sel + u*cand
                t = sm_pool.tile([128, D], f32, tag="o1")
                nc.vector.tensor_tensor(t[:], cand[:, kt, :], hsel[:, kt, :],
                                        op=SUB)
                t2 = sm_pool.tile([128, D], f32, tag="o2")
                nc.vector.tensor_tensor(t2[:], t[:], ru[:, kt, 64:128], op=MUL)
                nc.vector.tensor_tensor(out_sb[:, kt, :], t2[:],
                                        hsel[:, kt, :], op=ADD)
                nc.sync.dma_start(out_d[kt], out_sb[:, kt, :])

    if not nc.is_finalized():
        nc.finalize()
    return nc


def _get_graph():
    if "nc" not in _GRAPH_CACHE:
        _GRAPH_CACHE["nc"] = _build_graph()
    return _GRAPH_CACHE["nc"]


# ----------------------------------------------------------------------------
# host-side input prep
# ----------------------------------------------------------------------------
def _prep_in_maps(x, h, query_vectors, adj,
                  Wq, bq, Wk, bk, Wv, bv,
                  W_r, b_r, W_u, b_u, W_c, b_c):
    import ml_dtypes
    bf = ml_dtypes.bfloat16

    scale = 1.0 / np.sqrt(np.float32(C8))

    # packed per-head augmented projection weights: [130, Wq(16)|Wk(16)|Wv2(130)]
    wqkv = np.zeros((H, 130, 2 * C8 + 130), np.float32)
    for hh in range(H):
        wqkv[hh, 0:C, 0:C8] = Wq[hh] * scale
        wqkv[hh, C, 0:C8] = bq[hh] * scale
        wqkv[hh, 0:C, C8:2 * C8] = Wk[hh]
        wqkv[hh, C, C8:2 * C8] = bk[hh]
        wqkv[hh, 0:C, 2 * C8:2 * C8 + C] = Wv[hh]
        wqkv[hh, C, 2 * C8:2 * C8 + C] = bv[hh]
        wqkv[hh, C, 2 * C8 + C] = 1.0          # ones-column -> rowsum
    wqkv = np.ascontiguousarray(wqkv.transpose(1, 0, 2))      # [130, H, 162]

    # gate weights, flattened (q-major over (q, c)) for c = 0..127,
    # reshaped to the SBUF tile layout [128(c), 16(q), outdim]
    wru_flat = np.concatenate([W_r[:, 0:128, :], W_u[:, 0:128, :]], axis=2)
    wru_flat = np.ascontiguousarray(wru_flat.transpose(1, 0, 2))  # [128, 16, 128]
    wc_flat = np.ascontiguousarray(W_c[:, 0:128, :].transpose(1, 0, 2))
    m2b_ru = np.concatenate(
        [W_r[:, 128, :], W_u[:, 128, :], b_r, b_u], axis=1)       # [16, 256]
    m2b_c = np.concatenate([W_c[:, 128, :], b_c], axis=1)         # [16, 128]

    shared = {
        "wqkva": wqkv[0:128].astype(bf), "wqkvb": wqkv[128:130].astype(bf),
        "wru_flat": wru_flat.astype(bf), "wc_flat": wc_flat.astype(bf),
        "m2b_ru": m2b_ru.astype(bf), "m2b_c": m2b_c.astype(bf),
    }

    in_maps = []
    for core in range(NCORES):
        b, half = core // 2, core % 2
        n0 = half * NLOC
        g0 = b * N + n0

        combined = np.concatenate(
            [x[b], h[b], np.ones((N, 1), np.float32)], axis=1)    # [N, 130]
        combT = np.ascontiguousarray(combined.T)                  # [130, N]
        qvT = np.ascontiguousarray(query_vectors[g0:g0 + NLOC].T) # [16, 1024]
        # adjT[p, mt, k] = adj[b][n0+k, mt*128+p]
        adjT = np.ascontiguousarray(
            adj[b].T[:, n0:n0 + NLOC].reshape(16, 128, NLOC)
            .transpose(1, 0, 2)).astype(np.float32)               # [128,16,1024]
        qvrep = np.ascontiguousarray(
            np.broadcast_to(qvT[None, :, :], (128, Q, NLOC)))     # [128,16,1024]

        m = {
            "combT_a": combT[0:128].astype(bf),
            "combT_b": combT[128:130].astype(bf),
            "combTl_a": np.ascontiguousarray(combT[0:128, n0:n0 + NLOC]).astype(bf),
            "combTl_b": np.ascontiguousarray(combT[128:130, n0:n0 + NLOC]).astype(bf),
            "adjT": adjT.astype(bf),
            "qv_rep": qvrep.astype(bf),
            "qvT": qvT.astype(bf),
            "h_loc": np.ascontiguousarray(
                h[b, n0:n0 + NLOC].reshape(8, 128, D).transpose(1, 0, 2)),
            "h_locT": np.ascontiguousarray(h[b, n0:n0 + NLOC].T).astype(bf),
        }
        m.update(shared)
        in_maps.append(m)
    return in_maps


# ----------------------------------------------------------------------------
# entry point
# ----------------------------------------------------------------------------
def kernel(x, h, query_vectors, adj, nodes_flat,
           Wq, bq, Wk, bk, Wv, bv,
           W_r, b_r, W_u, b_u, W_c, b_c, _trace=False):
    args = dict(x=np.asarray(x, np.float32), h=np.asarray(h, np.float32),
                query_vectors=np.asarray(query_vectors, np.float32),
                adj=np.asarray(adj), nodes_flat=np.asarray(nodes_flat),
                Wq=np.asarray(Wq, np.float32), bq=np.asarray(bq, np.float32),
                Wk=np.asarray(Wk, np.float32), bk=np.asarray(bk, np.float32),
                Wv=np.asarray(Wv, np.float32), bv=np.asarray(bv, np.float32),
                W_r=np.asarray(W_r, np.float32), b_r=np.asarray(b_r, np.float32),
                W_u=np.asarray(W_u, np.float32), b_u=np.asarray(b_u, np.float32),
                W_c=np.asarray(W_c, np.float32), b_c=np.asarray(b_c, np.float32))

    if not np.array_equal(args["nodes_flat"].ravel(),
                          np.arange(K, dtype=np.int64)):
        return _numpy_reference(**args)

    from concourse.bass_utils import run_bass_kernel_spmd

    nc = _get_graph()
    in_maps = _prep_in_maps(
        args["x"], args["h"], args["query_vectors"], args["adj"],
        args["Wq"], args["bq"], args["Wk"], args["bk"], args["Wv"], args["bv"],
        args["W_r"], args["b_r"], args["W_u"], args["b_u"],
        args["W_c"], args["b_c"])

    res = run_bass_kernel_spmd(nc, in_maps, core_ids=list(range(NCORES)),
                               trace=_trace)
    out = np.concatenate(
        [np.asarray(res.results[i]["out"], np.float32).reshape(NLOC, D)
         for i in range(NCORES)], axis=0)
    if _trace:
        kernel.last_exec_time_ns = res.exec_time_ns
    return out


# revision 37
# speedup vs baseline: 1.3423x; 1.0520x over previous
"""Trainium2 Bass kernel for nn_AGATCellWithMLP (B=8,N=2048,D=64,Q=16,H=2,K=8192).

Sharding: nodes_flat == arange(8192) selects flattened rows 0..8191 == batches
0..3 only; attention for batches 4..7 never reaches the output.  8 cores =
4 batches x 2 n-halves (1024 output rows each), zero collectives.

Per-core pipeline (all matmuls bf16, accumulate f32 in PSUM):
  scores computed transposed  sT[m,n] = (k@qT)  so the softmax numerator p_T
  is directly the lhsT of attn@v; rowsum via a ones-column folded into v;
  leaky-relu+exp on ScalarE batched by activation-table set; adj mask on
  VectorE; hypernetwork gates via G[(q,c),k] = qv_T[q,k]*sel_T[c,k]
  outer-product matmuls, with the qv@b bias folded in as an extra
  contraction tile.  Every DRAM parameter is laid out host-side exactly as
  its SBUF destination so each input is one contiguous DMA.
"""

import numpy as np

B, N, D, Q, H = 8, 2048, 64, 16, 2
C = 2 * D + 1            # 129
C8 = 16
K = 8192
NLOC = 1024              # output rows per core
NCORES = 8
NEG = -9.0e15

_GRAPH_CACHE = {}


# ----------------------------------------------------------------------------
# numpy fallback (correct for arbitrary nodes_flat; slow)
# ----------------------------------------------------------------------------
def _numpy_reference(x, h, query_vectors, adj, nodes_flat,
                     Wq, bq, Wk, bk, Wv, bv,
                     W_r, b_r, W_u, b_u, W_c, b_c):
    x = x.astype(np.float32); h = h.astype(np.float32)
    combined = np.concatenate([x, h], axis=-1)
    q = np.einsum('bnc,hcd->hbnd', combined, Wq) + bq[:, None, None, :]
    k = np.einsum('bnc,hcd->hbnd', combined, Wk) + bk[:, None, None, :]
    v = np.einsum('bnc,hcd->hbnd', combined, Wv) + bv[:, None, None, :]
    comb_attn = np.zeros((B, N, C), np.float32)
    for b in range(B):
        acc = np.zeros((N, C), np.float32)
        for hh in range(H):
            s = (q[hh, b] @ k[hh, b].T) / np.sqrt(np.float32(C8))
            s = np.where(s >= 0, s, 0.2 * s)
            s = np.where(adj[b] == 0, NEG, s)
            s = s - s.max(axis=-1, keepdims=True)
            e = np.exp(s)
            a = e / e.sum(axis=-1, keepdims=True)
            acc += a @ v[hh, b]
        comb_attn[b] = acc / H
    def gate(sel, qv, W, bb):
        t = np.einsum('kc,qco->kqo', sel, W)
        return np.einsum('kq,kqo->ko', qv, t) + qv @ bb
    sel = comb_attn.reshape(-1, C)[nodes_flat]
    r = 1.0 / (1.0 + np.exp(-gate(sel, query_vectors, W_r, b_r)))
    u = 1.0 / (1.0 + np.exp(-gate(sel, query_vectors, W_u, b_u)))
    h_flat = h.reshape(-1, D).copy()
    h_sel = r * h_flat[nodes_flat]
    h_flat[nodes_flat] = h_sel
    comb_new = np.concatenate([x, h_flat.reshape(h.shape)], axis=-1)
    sel2 = comb_new.reshape(-1, C)[nodes_flat]
    cand = np.tanh(gate(sel2, query_vectors, W_c, b_c))
    return ((1.0 - u) * h_sel + u * cand).astype(np.float32)


# ----------------------------------------------------------------------------
# Bass graph builder (one SPMD graph, shapes per core)
# ----------------------------------------------------------------------------
def _build_graph():
    import concourse.bass as bass
    import concourse.bacc as bacc
    import concourse.mybir as mybir
    import concourse.tile as tile
    from concourse import masks
    from contextlib import ExitStack

    bf = mybir.dt.bfloat16
    f32 = mybir.dt.float32
    AF = mybir.ActivationFunctionType
    MUL = mybir.AluOpType.mult
    ADD = mybir.AluOpType.add
    SUB = mybir.AluOpType.subtract

    nc = bacc.Bacc(None, target_bir_lowering=False)

    # ---- DRAM parameters (exact SBUF layouts; one DMA each) -----------------
    wqkva_d  = nc.declare_dram_parameter("wqkva", [128, H, 2 * C8 + 130], bf, isOutput=False)
    wqkvb_d  = nc.declare_dram_parameter("wqkvb", [2, H, 2 * C8 + 130], bf, isOutput=False)
    combT_a  = nc.declare_dram_parameter("combT_a",  [128, N], bf, isOutput=False)
    combT_b  = nc.declare_dram_parameter("combT_b",  [2, N],   bf, isOutput=False)
    combTl_a = nc.declare_dram_parameter("combTl_a", [128, NLOC], bf, isOutput=False)
    combTl_b = nc.declare_dram_parameter("combTl_b", [2, NLOC],   bf, isOutput=False)
    qvT_d    = nc.declare_dram_parameter("qvT",      [Q, NLOC], bf, isOutput=False)
    hloc_d   = nc.declare_dram_parameter("h_loc",    [128, 8, D], f32, isOutput=False)
    hlocT_d  = nc.declare_dram_parameter("h_locT",   [D, NLOC], bf, isOutput=False)
    wru_d    = nc.declare_dram_parameter("wru_flat", [128, 16, 128], bf, isOutput=False)
    wc_d     = nc.declare_dram_parameter("wc_flat",  [128, 16, D], bf, isOutput=False)
    m2bru_d  = nc.declare_dram_parameter("m2b_ru",   [Q, 256], bf, isOutput=False)
    m2bc_d   = nc.declare_dram_parameter("m2b_c",    [Q, 128], bf, isOutput=False)
    adjT_d   = nc.declare_dram_parameter("adjT",     [128, 16, NLOC], bf, isOutput=False)
    qvrep_d  = nc.declare_dram_parameter("qv_rep",   [128, Q, NLOC], bf, isOutput=False)
    out_d    = nc.declare_dram_parameter("out",      [8, 128, D], f32, isOutput=True)

    with ExitStack() as ctx:
        tc = ctx.enter_context(tile.TileContext(nc))
        const = ctx.enter_context(tc.tile_pool(name="const", bufs=1))

        # ---- persistent SBUF tensors + input DMAs (critical first) ----------
        wqkva = const.tile([128, H, 2 * C8 + 130], bf)   # [Wq|Wk|Wv] aug rows 0..127
        wqkvb = const.tile([2, H, 2 * C8 + 130], bf)     # aug rows 128..129
        nc.sync.dma_start(wqkva[:], wqkva_d[:])
        nc.sync.dma_start(wqkvb[:], wqkvb_d[:])
        combTa = const.tile([128, N], bf)
        combTb = const.tile([2, N], bf)
        combTla = const.tile([128, NLOC], bf)
        combTlb = const.tile([2, NLOC], bf)
        nc.sync.dma_start(combTa[:], combT_a[:])
        nc.sync.dma_start(combTb[:], combT_b[:])
        nc.sync.dma_start(combTla[:], combTl_a[:])
        nc.sync.dma_start(combTlb[:], combTl_b[:])

        qvT = const.tile([Q, NLOC], bf)
        nc.sync.dma_start(qvT[:], qvT_d[:])
        hloc = const.tile([128, 8, D], f32)
        nc.sync.dma_start(hloc[:], hloc_d[:])
        hlocT = const.tile([D, NLOC], bf)
        nc.sync.dma_start(hlocT[:], hlocT_d[:])
        wru = const.tile([128, 16, 128], bf)
        nc.sync.dma_start(wru[:], wru_d[:])
        wc = const.tile([128, 16, D], bf)
        nc.sync.dma_start(wc[:], wc_d[:])
        m2bru = const.tile([Q, 256], bf)
        nc.sync.dma_start(m2bru[:], m2bru_d[:])
        m2bc = const.tile([Q, 128], bf)
        nc.sync.dma_start(m2bc[:], m2bc_d[:])

        # adjT and qv_rep share two 16KB/partition slots (disjoint lifetimes);
        # split in halves so the qv_rep DMA can start as soon as the first
        # half of the head-1 masks has consumed adjT.
        big_pool = ctx.enter_context(tc.tile_pool(name="big", bufs=1))
        adjT_h = []
        for bi in range(2):
            at = big_pool.tile([128, 8, NLOC], bf, tag=f"adjqv{bi}")
            nc.sync.dma_start(at[:], adjT_d[:, bi * 8:(bi + 1) * 8, :])
            adjT_h.append(at)

        def adjT(mt):
            return adjT_h[mt // 8][:, mt % 8, :]

        ident = const.tile([128, 128], f32)
        masks.make_identity(nc, ident[:])

        def wslice(hh, lo, hi):
            return wqkva[:, hh, lo:hi], wqkvb[:, hh, lo:hi]

        # ---- projections: qT, kT, v per head --------------------------------
        qTs = const.tile([C8, H, NLOC], bf)
        kTs = const.tile([C8, H, N], bf)
        vs  = const.tile([128, H, 16, 130], bf)

        with tc.tile_pool(name="proj_ps", bufs=2,
                          space=bass.MemorySpace.PSUM) as proj_ps:
            # PE warm-up burst: ~5us of back-to-back dummy matmuls while the
            # input DMAs land, so HAM unthrottles the PE clock (1.2->2.4GHz)
            # before the real matmuls start.
            wz = const.tile([128, 512], bf)
            nc.gpsimd.memset(wz[:], 0.0)
            for _ in range(12):
                wp = proj_ps.tile([128, 512], f32, tag="warm")
                nc.tensor.matmul(wp[:], wz[:, 0:128], wz[:],
                                 start=True, stop=True)
            # q/k projections first so the scores matmuls can start early
            for hh in range(H):
                wka_, wkb_ = wslice(hh, C8, 2 * C8)
                for kchk in range(2):
                    kcs = slice(kchk * NLOC, (kchk + 1) * NLOC)
                    kp = proj_ps.tile([C8, NLOC], f32, tag="kq")
                    for chk in range(2):
                        sl = slice(kchk * NLOC + chk * 512,
                                   kchk * NLOC + (chk + 1) * 512)
                        dl = slice(chk * 512, (chk + 1) * 512)
                        nc.tensor.matmul(kp[:, dl], wka_, combTa[:, sl],
                                         start=True, stop=False)
                        nc.tensor.matmul(kp[:, dl], wkb_, combTb[:, sl],
                                         start=False, stop=True)
                    nc.vector.tensor_copy(kTs[:, hh, kcs], kp[:])

                wqa_, wqb_ = wslice(hh, 0, C8)
                qp = proj_ps.tile([C8, NLOC], f32, tag="kq")
                for chk in range(2):
                    sl = slice(chk * 512, (chk + 1) * 512)
                    nc.tensor.matmul(qp[:, sl], wqa_, combTla[:, sl],
                                     start=True, stop=False)
                    nc.tensor.matmul(qp[:, sl], wqb_, combTlb[:, sl],
                                     start=False, stop=True)
                nc.vector.tensor_copy(qTs[:, hh, :], qp[:])

            pass

        # ---- attention per head --------------------------------------------
        hp0  = const.tile([128, 8, 130], f32)     # head-0: 0.5 * h' / rowsum
        comb = const.tile([128, 8, C], f32)       # mean over heads

        pT_pool = ctx.enter_context(tc.tile_pool(name="pT", bufs=1))
        lr_pool = ctx.enter_context(tc.tile_pool(name="lr", bufs=3))
        ex_pool = ctx.enter_context(tc.tile_pool(name="ex", bufs=3))
        sm_pool = ctx.enter_context(tc.tile_pool(name="small", bufs=2))
        selT = const.tile([128, 8, 128], bf)

        attn_ctx = ExitStack()
        sc_ps = attn_ctx.enter_context(
            tc.tile_pool(name="sc_ps", bufs=2, space=bass.MemorySpace.PSUM))
        hp_ps = attn_ctx.enter_context(
            tc.tile_pool(name="hp_ps", bufs=4, space=bass.MemorySpace.PSUM))

        # Per head, one fully-streamed loop per m-tile: scores (PE) ->
        # parametric-relu -> exp (both ScalarE, same activation-table set, so
        # zero table reloads) -> adj mask (DVE) -> 4 attn@v accumulator
        # matmuls + the v projection of a later tile (PE).  j 4..7 attn@v
        # runs as a dense burst after, overlapping the next head's stream.
        for hh in range(H):
            pT = pT_pool.tile([128, 16, NLOC], bf, tag="pT")
            wva_, wvb_ = wslice(hh, 2 * C8, 2 * C8 + 130)
            for mt in range(16):
                msl = slice(mt * 128, (mt + 1) * 128)
                vp = hp_ps.tile([128, 130], f32, tag="hp")
                nc.tensor.matmul(vp[:], combTa[:, msl], wva_,
                                 start=True, stop=False)
                nc.tensor.matmul(vp[:], combTb[:, msl], wvb_,
                                 start=False, stop=True)
                nc.vector.tensor_copy(vs[:, hh, mt, :], vp[:])
            accums = []
            for _j in range(4):
                acc_t = hp_ps.tile([128, 130], f32, tag="hp")
                accums.append(acc_t)
            for mt in range(16):
                msl = slice(mt * 128, (mt + 1) * 128)
                sp = sc_ps.tile([128, NLOC], f32, tag="s")
                for chk in range(2):
                    sl = slice(chk * 512, (chk + 1) * 512)
                    nc.tensor.matmul(sp[:, sl], kTs[:, hh, msl],
                                     qTs[:, hh, sl], start=True, stop=True)
                lr = lr_pool.tile([128, NLOC], bf, tag="lr")
                nc.scalar.activation(lr[:], sp[:], AF.Prelu, alpha=0.2)
                ex = ex_pool.tile([128, NLOC], bf, tag="ex")
                nc.scalar.activation(ex[:], lr[:], AF.Exp)
                nc.vector.tensor_tensor(pT[:, mt, :], ex[:], adjT(mt),
                                        op=MUL)
                for j in range(4):
                    jsl = slice(j * 128, (j + 1) * 128)
                    nc.tensor.matmul(accums[j][:], pT[:, mt, jsl],
                                     vs[:, hh, mt, :],
                                     start=(mt == 0), stop=(mt == 15))

            def _combine(hp_ap, j):
                rs = sm_pool.tile([128, 1], f32, tag="rs")
                nc.vector.reciprocal(rs[:], hp_ap[:, 129:130])
                if hh == 0:
                    nc.vector.tensor_scalar(hp0[:, j, 0:C], hp_ap[:, 0:C],
                                            rs[:], 0.5, op0=MUL, op1=MUL)
                else:
                    t1 = sm_pool.tile([128, C], f32, tag="t1")
                    nc.vector.tensor_scalar(t1[:], hp_ap[:, 0:C], rs[:], 0.5,
                                            op0=MUL, op1=MUL)
                    nc.vector.tensor_tensor(comb[:, j, :], hp0[:, j, 0:C],
                                            t1[:], op=ADD)
                    tpj = hp_ps.tile([128, 130], f32, tag="hp")
                    nc.tensor.transpose(tpj[:, 0:128], comb[:, j, 0:128],
                                        ident[:])
                    nc.vector.tensor_copy(selT[:, j, :], tpj[:, 0:128])

            for j in range(4):
                _combine(accums[j][:], j)
            for j in range(4, 8):
                jsl = slice(j * 128, (j + 1) * 128)
                hp = hp_ps.tile([128, 130], f32, tag="hp")
                for mt in range(16):
                    nc.tensor.matmul(hp[:], pT[:, mt, jsl], vs[:, hh, mt, :],
                                     start=(mt == 0), stop=(mt == 15))
                _combine(hp[:], j)

        attn_ctx.close()

        tp_ps = ctx.enter_context(
            tc.tile_pool(name="tp_ps", bufs=2, space=bass.MemorySpace.PSUM))

        def keep_warm(n=2):
            # HAM re-throttles the PE after ~3.4us of low activity; trickle
            # dummy matmuls into ACT/DVE-paced stretches to hold 2.4GHz.
            for _ in range(n):
                wp = tp_ps.tile([128, 128], f32, tag="tp")
                nc.tensor.matmul(wp[:], wz[:, 0:128], wz[:, 0:128],
                                 start=True, stop=True)

        # ---- gates r,u ------------------------------------------------------
        # qv_rep reuses adjT's half-slots (masks consumed them above)
        qvrep_h = []
        for bi in range(2):
            qr = big_pool.tile([128, 8, NLOC], bf, tag=f"adjqv{bi}")
            nc.sync.dma_start(qr[:], qvrep_d[:, bi * 8:(bi + 1) * 8, :])
            qvrep_h.append(qr)

        def qvrep(qt):
            return qvrep_h[qt // 8][:, qt % 8, :]

        G_pool = ctx.enter_context(tc.tile_pool(name="G", bufs=1))
        g_ps = ctx.enter_context(
            tc.tile_pool(name="g_ps", bufs=2, space=bass.MemorySpace.PSUM))
        m2_ps = ctx.enter_context(
            tc.tile_pool(name="m2_ps", bufs=2, space=bass.MemorySpace.PSUM))

        ru   = const.tile([128, 8, 128], f32)   # sigmoid outputs: r | u
        hsel = const.tile([128, 8, D], f32)
        rTb  = const.tile([D, 8, 128], bf)

        selT_flat = selT[:, :, :].rearrange("p a b -> p (a b)")
        for gc in range(2):
            gsl = slice(gc * 512, (gc + 1) * 512)
            G = G_pool.tile([128, Q, 512], bf, tag="G")
            for qt in range(Q):
                nc.vector.tensor_tensor(G[:, qt, :], selT_flat[:, gsl],
                                        qvrep(qt)[:, gsl], op=MUL)
            for kt in range(4 * gc, 4 * gc + 4):
                ksl = slice(kt * 128, (kt + 1) * 128)
                lsl = slice((kt - 4 * gc) * 128, (kt - 4 * gc + 1) * 128)
                gp = g_ps.tile([128, 128], f32, tag="g")
                for qt in range(Q):
                    nc.tensor.matmul(gp[:], G[:, qt, lsl], wru[:, qt, :],
                                     start=(qt == 0), stop=False)
                # bias qv@[b_r|b_u] as an extra contraction tile
                nc.tensor.matmul(gp[:], qvT[:, ksl], m2bru[:, 128:256],
                                 start=False, stop=True)
                m2 = m2_ps.tile([128, 128], f32, tag="m2")
                nc.tensor.matmul(m2[:], qvT[:, ksl], m2bru[:, 0:128],
                                 start=True, stop=True)
                a = sm_pool.tile([128, 128], f32, tag="ga")
                nc.vector.tensor_scalar(a[:], m2[:], comb[:, kt, 128:129],
                                        None, op0=MUL)
                pre = sm_pool.tile([128, 128], f32, tag="gp")
                nc.vector.tensor_tensor(pre[:], a[:], gp[:], op=ADD)
                nc.scalar.activation(ru[:, kt, :], pre[:], AF.Sigmoid)
                nc.vector.tensor_tensor(hsel[:, kt, :], ru[:, kt, 0:D],
                                        hloc[:, kt, :], op=MUL)
                # r transpose for sel2T, interleaved
                tp = tp_ps.tile([D, 128], f32, tag="tp")
                nc.tensor.transpose(tp[:], ru[:, kt, 0:D], ident[:])
                nc.vector.tensor_copy(rTb[:, kt, :], tp[:])

        # ---- gate c, pipelined per 512-chunk after ru ----------------------
        rTb_flat = rTb[:, :, :].rearrange("p a b -> p (a b)")
        rh = const.tile([D, NLOC], bf)
        sel2T = const.tile([128, NLOC], bf)
        cand = const.tile([128, 8, D], f32)
        out_sb = const.tile([128, 8, D], f32)
        for gc in range(2):
            gsl = slice(gc * 512, (gc + 1) * 512)
            # sel2T chunk: [x_T(65) | (r*h)T(63)]
            nc.vector.tensor_tensor(rh[:, gsl], rTb_flat[:, gsl],
                                    hlocT[:, gsl], op=MUL)
            nc.vector.tensor_copy(sel2T[0:65, gsl], combTla[0:65, gsl])
            nc.sync.dma_start(sel2T[65:128, gsl], rh[0:63, gsl])
            G2 = G_pool.tile([128, Q, 512], bf, tag="G")
            for qt in range(Q):
                nc.vector.tensor_tensor(G2[:, qt, :], sel2T[:, gsl],
                                        qvrep(qt)[:, gsl], op=MUL)
            for kt in range(4 * gc, 4 * gc + 4):
                ksl = slice(kt * 128, (kt + 1) * 128)
                lsl = slice((kt - 4 * gc) * 128, (kt - 4 * gc + 1) * 128)
                gp = g_ps.tile([128, D], f32, tag="g")
                for qt in range(Q):
                    nc.tensor.matmul(gp[:], G2[:, qt, lsl], wc[:, qt, :],
                                     start=(qt == 0), stop=False)
                nc.tensor.matmul(gp[:], qvT[:, ksl], m2bc[:, D:128],
                                 start=False, stop=True)
                m2 = m2_ps.tile([128, D], f32, tag="m2")
                nc.tensor.matmul(m2[:], qvT[:, ksl], m2bc[:, 0:D],
                                 start=True, stop=True)
                a = sm_pool.tile([128, D], f32, tag="ca")
                # sel2 col 128 is h_sel[:, 63]
                nc.vector.tensor_scalar(a[:], m2[:], hsel[:, kt, 63:64], None,
                                        op0=MUL)
                pre = sm_pool.tile([128, D], f32, tag="cp")
                nc.vector.tensor_tensor(pre[:], a[:], gp[:], op=ADD)
                nc.scalar.activation(cand[:, kt, :], pre[:], AF.Tanh)
                # output: (1-u)*h_sel + u*cand
                t = sm_pool.tile([128, D], f32, tag="o1")
                nc.vector.tensor_tensor(t[:], cand[:, kt, :], hsel[:, kt, :],
                                        op=SUB)
                t2 = sm_pool.tile([128, D], f32, tag="o2")
                nc.vector.tensor_tensor(t2[:], t[:], ru[:, kt, 64:128], op=MUL)
                nc.vector.tensor_tensor(out_sb[:, kt, :], t2[:],
                                        hsel[:, kt, :], op=ADD)
                nc.sync.dma_start(out_d[kt], out_sb[:, kt, :])

    if not nc.is_finalized():
        nc.finalize()
    return nc


def _get_graph():
    if "nc" not in _GRAPH_CACHE:
        _GRAPH_CACHE["nc"] = _build_graph()
    return _GRAPH_CACHE["nc"]


# ----------------------------------------------------------------------------
# host-side input prep
# ----------------------------------------------------------------------------
def _prep_in_maps(x, h, query_vectors, adj,
                  Wq, bq, Wk, bk, Wv, bv,
                  W_r, b_r, W_u, b_u, W_c, b_c):
    import ml_dtypes
    bf = ml_dtypes.bfloat16

    scale = 1.0 / np.sqrt(np.float32(C8))

    # packed per-head augmented projection weights: [130, Wq(16)|Wk(16)|Wv2(130)]
    wqkv = np.zeros((H, 130, 2 * C8 + 130), np.float32)
    for hh in range(H):
        wqkv[hh, 0:C, 0:C8] = Wq[hh] * scale
        wqkv[hh, C, 0:C8] = bq[hh] * scale
        wqkv[hh, 0:C, C8:2 * C8] = Wk[hh]
        wqkv[hh, C, C8:2 * C8] = bk[hh]
        wqkv[hh, 0:C, 2 * C8:2 * C8 + C] = Wv[hh]
        wqkv[hh, C, 2 * C8:2 * C8 + C] = bv[hh]
        wqkv[hh, C, 2 * C8 + C] = 1.0          # ones-column -> rowsum
    wqkv = np.ascontiguousarray(wqkv.transpose(1, 0, 2))      # [130, H, 162]

    # gate weights, flattened (q-major over (q, c)) for c = 0..127,
    # reshaped to the SBUF tile layout [128(c), 16(q), outdim]
    wru_flat = np.concatenate([W_r[:, 0:128, :], W_u[:, 0:128, :]], axis=2)
    wru_flat = np.ascontiguousarray(wru_flat.transpose(1, 0, 2))  # [128, 16, 128]
    wc_flat = np.ascontiguousarray(W_c[:, 0:128, :].transpose(1, 0, 2))
    m2b_ru = np.concatenate(
        [W_r[:, 128, :], W_u[:, 128, :], b_r, b_u], axis=1)       # [16, 256]
    m2b_c = np.concatenate([W_c[:, 128, :], b_c], axis=1)         # [16, 128]

    shared = {
        "wqkva": wqkv[0:128].astype(bf), "wqkvb": wqkv[128:130].astype(bf),
        "wru_flat": wru_flat.astype(bf), "wc_flat": wc_flat.astype(bf),
        "m2b_ru": m2b_ru.astype(bf), "m2b_c": m2b_c.astype(bf),
    }

    in_maps = []
    for core in range(NCORES):
        b, half = core // 2, core % 2
        n0 = half * NLOC
        g0 = b * N + n0

        combined = np.concatenate(
            [x[b], h[b], np.ones((N, 1), np.float32)], axis=1)    # [N, 130]
        combT = np.ascontiguousarray(combined.T)                  # [130, N]
        qvT = np.ascontiguousarray(query_vectors[g0:g0 + NLOC].T) # [16, 1024]
        # adjT[p, mt, k] = adj[b][n0+k, mt*128+p]
        adjT = np.ascontiguousarray(
            adj[b].T[:, n0:n0 + NLOC].reshape(16, 128, NLOC)
            .transpose(1, 0, 2)).astype(np.float32)               # [128,16,1024]
        qvrep = np.ascontiguousarray(
            np.broadcast_to(qvT[None, :, :], (128, Q, NLOC)))     # [128,16,1024]

        m = {
            "combT_a": combT[0:128].astype(bf),
            "combT_b": combT[128:130].astype(bf),
            "combTl_a": np.ascontiguousarray(combT[0:128, n0:n0 + NLOC]).astype(bf),
            "combTl_b": np.ascontiguousarray(combT[128:130, n0:n0 + NLOC]).astype(bf),
            "adjT": adjT.astype(bf),
            "qv_rep": qvrep.astype(bf),
            "qvT": qvT.astype(bf),
            "h_loc": np.ascontiguousarray(
                h[b, n0:n0 + NLOC].reshape(8, 128, D).transpose(1, 0, 2)),
            "h_locT": np.ascontiguousarray(h[b, n0:n0 + NLOC].T).astype(bf),
        }
        m.update(shared)
        in_maps.append(m)
    return in_maps


# ----------------------------------------------------------------------------
# entry point
# ----------------------------------------------------------------------------
def kernel(x, h, query_vectors, adj, nodes_flat,
           Wq, bq, Wk, bk, Wv, bv,
           W_r, b_r, W_u, b_u, W_c, b_c, _trace=False):
    args = dict(x=np.asarray(x, np.float32), h=np.asarray(h, np.float32),
                query_vectors=np.asarray(query_vectors, np.float32),
                adj=np.asarray(adj), nodes_flat=np.asarray(nodes_flat),
                Wq=np.asarray(Wq, np.float32), bq=np.asarray(bq, np.float32),
                Wk=np.asarray(Wk, np.float32), bk=np.asarray(bk, np.float32),
                Wv=np.asarray(Wv, np.float32), bv=np.asarray(bv, np.float32),
                W_r=np.asarray(W_r, np.float32), b_r=np.asarray(b_r, np.float32),
                W_u=np.asarray(W_u, np.float32), b_u=np.asarray(b_u, np.float32),
                W_c=np.asarray(W_c, np.float32), b_c=np.asarray(b_c, np.float32))

    if not np.array_equal(args["nodes_flat"].ravel(),
                          np.arange(K, dtype=np.int64)):
        return _numpy_reference(**args)

    from concourse.bass_utils import run_bass_kernel_spmd

    nc = _get_graph()
    in_maps = _prep_in_maps(
        args["x"], args["h"], args["query_vectors"], args["adj"],
        args["Wq"], args["bq"], args["Wk"], args["bk"], args["Wv"], args["bv"],
        args["W_r"], args["b_r"], args["W_u"], args["b_u"],
        args["W_c"], args["b_c"])

    res = run_bass_kernel_spmd(nc, in_maps, core_ids=list(range(NCORES)),
                               trace=_trace)
    out = np.concatenate(
        [np.asarray(res.results[i]["out"], np.float32).reshape(NLOC, D)
         for i in range(NCORES)], axis=0)
    if _trace:
        kernel.last_exec_time_ns = res.exec_time_ns
    return out
